# revision 5
# baseline (speedup 1.0000x reference)
"""Trainium2 Bass kernel for LocalLuongAttention.

reference semantics (B=32, S=4096, D=1024, O=1024, STDDEV=8):
    score[b,s]  = sum_d src[b,s,d] * tgt[b,d]
    weights     = softmax(score, axis=1) * exp(-(s-pos[b])^2 / (2*8^2))
    weighted[b] = sum_s weights[b,s] * src[b,s,:]
    out         = tanh(concat([tgt, weighted], 1) @ W)        # W: [2048, 1024]

Distribution: data-parallel over batch, 4 batches per core on 8 cores, W
replicated, no collectives.

The Gaussian position decay is <= exp(-32) ~ 1.3e-14 outside +/-64 of pos,
so the weighted sum only needs a 256-row window of src (sliced on host,
kept fp32).  The full src still streams through the chip once for the
softmax normalizer -- but only in bf16 (half the HBM traffic; the
normalizer is dominated by the top scores and tolerates the rounding) and
in host-transposed [D, S] layout so the dot products run on the otherwise
idle PE array (stationary tgt column, streaming src rows) instead of the
vector engine.  Scores for one batch accumulate over 8 d-chunks into
[1, 512] PSUM tiles and land on partition 0 as a [1, 4096] row, where the
softmax max/normalizer are single-partition reductions.
"""

import sys

for _p in ("/opt/trn_rl_repo",):
    if _p not in sys.path:
        sys.path.insert(0, _p)

from contextlib import ExitStack

import ml_dtypes
import numpy as np

import concourse.bass as bass
import concourse.tile as tile
from concourse import bacc, bass_isa, mybir
from concourse._compat import with_exitstack
from concourse.bass_utils import run_bass_kernel_spmd

B, S, D, O = 32, 4096, 1024, 1024
STDDEV = 8.0
N_CORES = 8
BPC = B // N_CORES   # batches per core
WIN = 256            # window rows kept fp32 for the weighted sum
HALF = 64            # guaranteed covered half-window
KC = (2 * D) // 128  # 16 contraction chunks of the projection
KD = D // 128        # 8 d-chunks of the score contraction
SH = 2048            # sequence columns per streamed tile ([128, SH] bf16)
NB = SH // 512       # 512-wide matmul blocks per streamed tile

FP32 = mybir.dt.float32
BF16 = mybir.dt.bfloat16

_CACHE = {}
LAST_RESULTS = None  # BassKernelResults of the most recent run


def _install_ntff_shim():
    """Register the NTFF profile hook that this image's antenv lacks.

    Drives profiling via ctypes into libaxon_pjrt.so (same mechanism the
    full antenv.axon_hooks module uses) and stubs out the artifact upload.
    Only needed for trace=True runs.
    """
    import contextlib
    import ctypes
    import types

    if "antenv.axon_hooks" in sys.modules:
        return
    lib = ctypes.CDLL("/opt/axon/libaxon_pjrt.so")
    if not hasattr(lib, "axon_start_nrt_profile"):
        raise RuntimeError("libaxon_pjrt.so lacks profile symbols")
    lib.axon_start_nrt_profile.argtypes = [
        ctypes.POINTER(ctypes.c_int64), ctypes.c_size_t]
    lib.axon_start_nrt_profile.restype = ctypes.c_int64
    lib.axon_stop_nrt_profile.argtypes = [ctypes.c_char_p]
    lib.axon_stop_nrt_profile.restype = ctypes.c_int64

    @contextlib.contextmanager
    def _hook(output_dir, device_ids):
        import jax
        jax.devices()
        if device_ids:
            ids = (ctypes.c_int64 * len(device_ids))(*device_ids)
            rc = lib.axon_start_nrt_profile(ids, len(device_ids))
        else:
            rc = lib.axon_start_nrt_profile(None, 0)
        if rc != 0:
            raise RuntimeError(f"axon_start_nrt_profile rc={rc}")
        try:
            yield
        finally:
            n = lib.axon_stop_nrt_profile(str(output_dir).encode())
            print(f"ntff profile: {n} file(s) -> {output_dir}",
                  file=sys.stderr)

    m = types.ModuleType("antenv.axon_hooks")
    m.get_axon_ntff_profile_hook = lambda: _hook
    m.set_axon_ntff_profile_hook = lambda h: None
    sys.modules["antenv.axon_hooks"] = m
    import concourse.bass_utils as _bu
    _bu.upload_artifacts = lambda tmpdir: f"local://{tmpdir}"


@with_exitstack
def _body(ctx: ExitStack, tc: tile.TileContext, out, srcT, tgt16t, tgt,
          tgt_t, srcwin, logpw, wmat):
    nc = tc.nc
    mult = mybir.AluOpType.mult
    maxop = mybir.AluOpType.max
    Exp = mybir.ActivationFunctionType.Exp
    Tanh = mybir.ActivationFunctionType.Tanh

    consts = ctx.enter_context(tc.tile_pool(name="consts", bufs=1))
    wpool = ctx.enter_context(tc.tile_pool(name="wpool", bufs=1))
    tgtbp = ctx.enter_context(tc.tile_pool(name="tgtb", bufs=2))
    srcp = ctx.enter_context(tc.tile_pool(name="srcp", bufs=8))
    winp = ctx.enter_context(tc.tile_pool(name="winp", bufs=2))
    scp = ctx.enter_context(tc.tile_pool(name="scores", bufs=2))
    stats = ctx.enter_context(tc.tile_pool(name="stats", bufs=4))
    outp = ctx.enter_context(tc.tile_pool(name="outp", bufs=2))
    psc = ctx.enter_context(tc.tile_pool(name="psc", bufs=4, space="PSUM"))
    psw = ctx.enter_context(tc.tile_pool(name="psw", bufs=2, space="PSUM"))
    pso = ctx.enter_context(tc.tile_pool(name="pso", bufs=1, space="PSUM"))

    # Stationary tgt columns for the score matmuls: [128, d_chunk, batch]
    tg16 = consts.tile([128, KD, BPC], BF16)
    nc.sync.dma_start(out=tg16, in_=tgt16t.rearrange("(c p) b -> p c b",
                                                     p=128))

    # Resident projection weights: [128, k_chunk, O] (8 MB, 4 x 2MB DMAs)
    wsb = wpool.tile([128, KC, O], FP32)
    wre = wmat.rearrange("(k p) d -> p k d", p=128)
    for j in range(4):
        nc.sync.dma_start(out=wsb[:, 4 * j:4 * (j + 1), :],
                          in_=wre[:, 4 * j:4 * (j + 1), :])

    # combined.T laid out [128, k_chunk, batch]; chunks 0..7 are tgt.T
    # (from host), chunks 8..15 get weighted.T from the matmuls below.
    combT = consts.tile([128, KC, BPC], FP32)
    tre = tgt_t.rearrange("(k p) b -> p k b", p=128)
    nc.sync.dma_start(out=combT[:, 0:KC // 2, :], in_=tre)

    po = [pso.tile([BPC, 512], FP32, name=f"po{h}", tag=f"po{h}")
          for h in range(2)]

    scr = consts.tile([128, D], FP32)   # discarded STT elementwise output
    zdisc = consts.tile([1, S], FP32)   # discarded exp output

    for b in range(BPC):
        # --- window path (fp32, on DVE) -------------------------------
        tgtr = tgtbp.tile([1, D], FP32, tag="tgtr")
        nc.sync.dma_start(out=tgtr, in_=tgt[b:b + 1, :])
        tgtb = tgtbp.tile([128, D], FP32)
        nc.gpsimd.partition_broadcast(tgtb, tgtr)

        winsb = winp.tile([128, 2, D], FP32)
        nc.sync.dma_start(out=winsb,
                          in_=srcwin[b].rearrange("(t p) d -> p t d", p=128))
        wsc = stats.tile([128, 2], FP32)
        for t in range(2):
            nc.vector.scalar_tensor_tensor(
                out=scr, in0=winsb[:, t, :], scalar=0.0, in1=tgtb,
                op0=mybir.AluOpType.bypass, op1=mult,
                accum_out=wsc[:, t:t + 1])
        lpw = stats.tile([128, 2], FP32)
        nc.sync.dma_start(out=lpw, in_=logpw[b])

        # --- bf16 score stream on the PE ------------------------------
        # scores[0, s] = sum_d srcT[d, s] * tgt[d], accumulated over the
        # 8 d-chunks into [1, 512] PSUM tiles.
        scores = scp.tile([1, S], FP32)
        for h in range(S // SH):
            ps = [psc.tile([1, 512], FP32, name=f"ps{j}", tag="ps")
                  for j in range(NB)]
            for c in range(KD):
                st = srcp.tile([128, SH], BF16)
                nc.sync.dma_start(
                    out=st,
                    in_=srcT[b, 128 * c:128 * (c + 1), SH * h:SH * (h + 1)])
                for j in range(NB):
                    nc.tensor.matmul(ps[j], lhsT=tg16[:, c, b:b + 1],
                                     rhs=st[:, 512 * j:512 * (j + 1)],
                                     start=(c == 0), stop=(c == KD - 1),
                                     skip_group_check=True)
            for j in range(NB):
                nc.vector.tensor_copy(
                    scores[:, SH * h + 512 * j:SH * h + 512 * (j + 1)],
                    ps[j])

        if b == 0:
            # tgt half of the projection: PE is free while batch 0's
            # stats resolve; accumulation groups stay open to the end.
            for hh in range(2):
                for k in range(KC // 2):
                    nc.tensor.matmul(po[hh], lhsT=combT[:, k, :],
                                     rhs=wsb[:, k, 512 * hh:512 * (hh + 1)],
                                     start=(k == 0), stop=False,
                                     skip_group_check=True)

        # --- softmax stats on the [1, 4096] score row -----------------
        m1 = stats.tile([1, 1], FP32)
        nc.vector.tensor_reduce(m1, scores, mybir.AxisListType.X, maxop)
        negm = stats.tile([1, 1], FP32)
        nc.vector.tensor_scalar_mul(negm, m1, -1.0)
        zp = stats.tile([1, 1], FP32)
        nc.scalar.activation(zdisc, scores, Exp, bias=negm, accum_out=zp)
        rz = stats.tile([1, 1], FP32)
        nc.vector.reciprocal(rz, zp)
        negmb = stats.tile([128, 1], FP32)
        nc.gpsimd.partition_broadcast(negmb, negm)
        rzb = stats.tile([128, 1], FP32)
        nc.gpsimd.partition_broadcast(rzb, rz)

        # window weights: exp(score + logpw - m) / Z
        wpre = stats.tile([128, 2], FP32)
        nc.vector.tensor_add(wpre, wsc, lpw)
        wexp = stats.tile([128, 2], FP32)
        nc.scalar.activation(wexp, wpre, Exp, bias=negmb)
        wfin = stats.tile([128, 2], FP32)
        nc.vector.tensor_scalar_mul(wfin, wexp, rzb)

        # weighted.T chunks: contract window rows on the PE
        for c in range(8):
            pw = psw.tile([128, 1], FP32)
            nc.tensor.matmul(pw, lhsT=winsb[:, 0, 128 * c:128 * (c + 1)],
                             rhs=wfin[:, 0:1], start=True, stop=False)
            nc.tensor.matmul(pw, lhsT=winsb[:, 1, 128 * c:128 * (c + 1)],
                             rhs=wfin[:, 1:2], start=False, stop=True)
            nc.vector.tensor_copy(combT[:, KC // 2 + c, b:b + 1], pw)

    # weighted half of the projection closes the accumulation groups
    for hh in range(2):
        for k in range(KC // 2, KC):
            nc.tensor.matmul(po[hh], lhsT=combT[:, k, :],
                             rhs=wsb[:, k, 512 * hh:512 * (hh + 1)],
                             start=False, stop=(k == KC - 1),
                             skip_group_check=True)
        ot = outp.tile([BPC, 512], FP32)
        nc.scalar.activation(ot, po[hh], Tanh)
        nc.sync.dma_start(out=out[:, 512 * hh:512 * (hh + 1)], in_=ot)


def build():
    if "nc" in _CACHE:
        return _CACHE["nc"]
    nc = bacc.Bacc("TRN2", target_bir_lowering=False, debug=False,
                   enable_asserts=False, num_devices=N_CORES)
    srcT = nc.dram_tensor("srcT", [BPC, D, S], BF16, kind="ExternalInput").ap()
    tgt16t = nc.dram_tensor("tgt16t", [D, BPC], BF16,
                            kind="ExternalInput").ap()
    tgt = nc.dram_tensor("tgt", [BPC, D], FP32, kind="ExternalInput").ap()
    tgt_t = nc.dram_tensor("tgt_t", [D, BPC], FP32, kind="ExternalInput").ap()
    srcwin = nc.dram_tensor("srcwin", [BPC, WIN, D], FP32,
                            kind="ExternalInput").ap()
    logpw = nc.dram_tensor("logpw", [BPC, 128, 2], FP32,
                           kind="ExternalInput").ap()
    wmat = nc.dram_tensor("wmat", [2 * D, O], FP32, kind="ExternalInput").ap()
    out = nc.dram_tensor("out", [BPC, O], FP32, kind="ExternalOutput").ap()
    with tile.TileContext(nc) as tc:
        _body(tc, out, srcT, tgt16t, tgt, tgt_t, srcwin, logpw, wmat)
    nc.compile()
    _CACHE["nc"] = nc
    return nc


def make_in_maps(src, tgt, pos, wmat):
    """Host-side sharding + bf16 transpose + window/log-posweight precompute."""
    src16 = src.astype(ml_dtypes.bfloat16)
    w0 = np.clip(128 * ((pos.astype(np.int64) - HALF) // 128), 0, S - WIN)
    p_idx = np.arange(128, dtype=np.int64)[:, None]
    t_idx = np.arange(2, dtype=np.int64)[None, :]
    in_maps = []
    for c in range(N_CORES):
        bsl = slice(c * BPC, (c + 1) * BPC)
        srcwin = np.stack([
            src[c * BPC + i, w0[c * BPC + i]:w0[c * BPC + i] + WIN, :]
            for i in range(BPC)
        ])
        logpw = np.stack([
            -((w0[c * BPC + i] + t_idx * 128 + p_idx
               - pos[c * BPC + i]).astype(np.float64) ** 2)
            / (2.0 * STDDEV * STDDEV)
            for i in range(BPC)
        ]).astype(np.float32)
        in_maps.append({
            "srcT": np.ascontiguousarray(src16[bsl].transpose(0, 2, 1)),
            "tgt16t": np.ascontiguousarray(
                tgt[bsl].T.astype(ml_dtypes.bfloat16)),
            "tgt": np.ascontiguousarray(tgt[bsl]),
            "tgt_t": np.ascontiguousarray(tgt[bsl].T),
            "srcwin": np.ascontiguousarray(srcwin),
            "logpw": logpw,
            "wmat": wmat,
        })
    return in_maps


def kernel(source_hidden_sequence, target_hidden, positions,
           attention_weights, trace=False):
    src = np.ascontiguousarray(source_hidden_sequence, dtype=np.float32)
    tgt = np.ascontiguousarray(target_hidden, dtype=np.float32)
    pos = np.asarray(positions)
    wmat = np.ascontiguousarray(attention_weights, dtype=np.float32)
    assert src.shape == (B, S, D) and wmat.shape == (2 * D, O)

    nc = build()
    if trace:
        _install_ntff_shim()
    in_maps = make_in_maps(src, tgt, pos, wmat)
    res = run_bass_kernel_spmd(nc, in_maps, list(range(N_CORES)), trace=trace)
    global LAST_RESULTS
    LAST_RESULTS = res
    out = np.concatenate([res.results[c]["out"] for c in range(N_CORES)],
                         axis=0)
    return out.astype(np.float32)


# revision 19
# speedup vs baseline: 1.0130x; 1.0130x over previous
"""Trainium2 Bass kernel for LocalLuongAttention.

reference semantics (B=32, S=4096, D=1024, O=1024, STDDEV=8):
    score[b,s]  = sum_d src[b,s,d] * tgt[b,d]
    weights     = softmax(score, axis=1) * exp(-(s-pos[b])^2 / (2*8^2))
    weighted[b] = sum_s weights[b,s] * src[b,s,:]
    out         = tanh(concat([tgt, weighted], 1) @ W)        # W: [2048, 1024]

Distribution: data-parallel over batch, 4 batches per core on 8 cores, W
replicated, no collectives.

The Gaussian position decay is <= exp(-32) ~ 1.3e-14 outside +/-64 of pos,
so the weighted sum only needs a 256-row window of src (sliced on host,
kept fp32).  The full src still streams through the chip once for the
softmax normalizer -- but only in bf16 (half the HBM traffic; the
normalizer is dominated by the top scores and tolerates the rounding) and
in host-transposed [D, S] layout so the dot products run on the otherwise
idle PE array (stationary tgt column, streaming src rows) instead of the
vector engine.  Scores for one batch accumulate over 8 d-chunks into
[1, 512] PSUM tiles and land on partition 0 as a [1, 4096] row, where the
softmax max/normalizer are single-partition reductions.
"""

import sys

for _p in ("/opt/trn_rl_repo",):
    if _p not in sys.path:
        sys.path.insert(0, _p)

from contextlib import ExitStack

import ml_dtypes
import numpy as np

import concourse.bass as bass
import concourse.tile as tile
from concourse import bacc, bass_isa, mybir
from concourse._compat import with_exitstack
from concourse.bass_utils import run_bass_kernel_spmd

B, S, D, O = 32, 4096, 1024, 1024
STDDEV = 8.0
N_CORES = 8
BPC = B // N_CORES   # batches per core
WIN = 256            # window rows kept fp32 for the weighted sum
HALF = 64            # guaranteed covered half-window
KC = (2 * D) // 128  # 16 contraction chunks of the projection
KD = D // 128        # 8 d-chunks of the score contraction
SH = 2048            # sequence columns per streamed tile ([128, SH] bf16)
NB = SH // 512       # 512-wide matmul blocks per streamed tile

FP32 = mybir.dt.float32
FP32R = mybir.dt.float32r  # PE single-pass fp32 mode: 1 cycle/row at N>=256
BF16 = mybir.dt.bfloat16

_CACHE = {}
LAST_RESULTS = None  # BassKernelResults of the most recent run


def _install_ntff_shim():
    """Register the NTFF profile hook that this image's antenv lacks.

    Drives profiling via ctypes into libaxon_pjrt.so (same mechanism the
    full antenv.axon_hooks module uses) and stubs out the artifact upload.
    Only needed for trace=True runs.
    """
    import contextlib
    import ctypes
    import types

    if "antenv.axon_hooks" in sys.modules:
        return
    lib = ctypes.CDLL("/opt/axon/libaxon_pjrt.so")
    if not hasattr(lib, "axon_start_nrt_profile"):
        raise RuntimeError("libaxon_pjrt.so lacks profile symbols")
    lib.axon_start_nrt_profile.argtypes = [
        ctypes.POINTER(ctypes.c_int64), ctypes.c_size_t]
    lib.axon_start_nrt_profile.restype = ctypes.c_int64
    lib.axon_stop_nrt_profile.argtypes = [ctypes.c_char_p]
    lib.axon_stop_nrt_profile.restype = ctypes.c_int64

    @contextlib.contextmanager
    def _hook(output_dir, device_ids):
        import jax
        jax.devices()
        if device_ids:
            ids = (ctypes.c_int64 * len(device_ids))(*device_ids)
            rc = lib.axon_start_nrt_profile(ids, len(device_ids))
        else:
            rc = lib.axon_start_nrt_profile(None, 0)
        if rc != 0:
            raise RuntimeError(f"axon_start_nrt_profile rc={rc}")
        try:
            yield
        finally:
            n = lib.axon_stop_nrt_profile(str(output_dir).encode())
            print(f"ntff profile: {n} file(s) -> {output_dir}",
                  file=sys.stderr)

    m = types.ModuleType("antenv.axon_hooks")
    m.get_axon_ntff_profile_hook = lambda: _hook
    m.set_axon_ntff_profile_hook = lambda h: None
    sys.modules["antenv.axon_hooks"] = m
    import concourse.bass_utils as _bu
    _bu.upload_artifacts = lambda tmpdir: f"local://{tmpdir}"


@with_exitstack
def _body(ctx: ExitStack, tc: tile.TileContext, out, srcT, tgt16t, tgt,
          tgt_t, srcwin, logpw, wmat):
    nc = tc.nc
    mult = mybir.AluOpType.mult
    maxop = mybir.AluOpType.max
    Exp = mybir.ActivationFunctionType.Exp
    Tanh = mybir.ActivationFunctionType.Tanh

    consts = ctx.enter_context(tc.tile_pool(name="consts", bufs=1))
    wpool = ctx.enter_context(tc.tile_pool(name="wpool", bufs=1))
    tgtbp = ctx.enter_context(tc.tile_pool(name="tgtb", bufs=2))
    srcp = ctx.enter_context(tc.tile_pool(name="srcp", bufs=4))
    winp = ctx.enter_context(tc.tile_pool(name="winp", bufs=2))
    scp = ctx.enter_context(tc.tile_pool(name="scores", bufs=1))
    stats = ctx.enter_context(tc.tile_pool(name="stats", bufs=4))
    outp = ctx.enter_context(tc.tile_pool(name="outp", bufs=2))
    psc = ctx.enter_context(tc.tile_pool(name="psc", bufs=4, space="PSUM"))
    psw = ctx.enter_context(tc.tile_pool(name="psw", bufs=2, space="PSUM"))
    pso = ctx.enter_context(tc.tile_pool(name="pso", bufs=1, space="PSUM"))

    # Stationary tgt columns for the score matmuls: [128, d_chunk, batch]
    tg16 = consts.tile([128, KD, BPC], BF16)
    nc.sync.dma_start(out=tg16, in_=tgt16t.rearrange("(c p) b -> p c b",
                                                     p=128))

    # Resident projection weights: [128, k_chunk, O] (8 MB, 4 x 2MB DMAs)
    wsb = wpool.tile([128, KC, O], FP32)
    wre = wmat.rearrange("(k p) d -> p k d", p=128)
    for j in range(4):
        nc.sync.dma_start(out=wsb[:, 4 * j:4 * (j + 1), :],
                          in_=wre[:, 4 * j:4 * (j + 1), :])

    # combined.T laid out [128, k_chunk, batch]; chunks 0..7 are tgt.T
    # (from host), chunks 8..15 get weighted.T from the matmuls below.
    combT = consts.tile([128, KC, BPC], FP32)
    tre = tgt_t.rearrange("(k p) b -> p k b", p=128)
    nc.sync.dma_start(out=combT[:, 0:KC // 2, :], in_=tre)

    po = [pso.tile([BPC, 512], FP32, name=f"po{h}", tag=f"po{h}")
          for h in range(2)]

    scr = consts.tile([128, D], FP32)   # discarded STT elementwise output
    zdisc = consts.tile([1, S], FP32)   # discarded exp output

    for b in range(BPC):
        # --- window path (fp32, on DVE) -------------------------------
        tgtr = tgtbp.tile([1, D], FP32, tag="tgtr")
        nc.sync.dma_start(out=tgtr, in_=tgt[b:b + 1, :])
        tgtb = tgtbp.tile([128, D], FP32)
        nc.gpsimd.partition_broadcast(tgtb, tgtr)

        winsb = winp.tile([128, 2, D], FP32)
        nc.sync.dma_start(out=winsb,
                          in_=srcwin[b].rearrange("(t p) d -> p t d", p=128))
        wsc = stats.tile([128, 2], FP32)
        for t in range(2):
            nc.vector.scalar_tensor_tensor(
                out=scr, in0=winsb[:, t, :], scalar=0.0, in1=tgtb,
                op0=mybir.AluOpType.bypass, op1=mult,
                accum_out=wsc[:, t:t + 1])
        lpw = stats.tile([128, 2], FP32)
        nc.sync.dma_start(out=lpw, in_=logpw[b])

        # --- bf16 score stream on the PE ------------------------------
        # scores[0, s] = sum_d srcT[d, s] * tgt[d], accumulated over the
        # 8 d-chunks into [1, 512] PSUM tiles.  Each 2 MB DMA carries 4
        # d-chunks of one s-half; block maxes (m8) start as soon as each
        # [1, 512] block lands so only the last one is on the tail.
        scores = scp.tile([1, S], FP32)
        m8 = stats.tile([1, 2 * NB], FP32)
        srcr = srcT[b].rearrange("(g p) s -> p g s", p=128)
        for h in range(S // SH):
            ps = [psc.tile([1, 512], FP32, name=f"ps{j}", tag="ps")
                  for j in range(NB)]
            for q in range(2):
                st = srcp.tile([128, KD // 2, SH], BF16)
                nc.sync.dma_start(
                    out=st,
                    in_=srcr[:, 4 * q:4 * (q + 1), SH * h:SH * (h + 1)])
                for g in range(KD // 2):
                    c = 4 * q + g
                    for j in range(NB):
                        nc.tensor.matmul(ps[j], lhsT=tg16[:, c, b:b + 1],
                                         rhs=st[:, g, 512 * j:512 * (j + 1)],
                                         start=(c == 0), stop=(c == KD - 1),
                                         skip_group_check=True)
            for j in range(NB):
                sl = scores[:, SH * h + 512 * j:SH * h + 512 * (j + 1)]
                nc.vector.tensor_copy(sl, ps[j])
                nc.vector.tensor_reduce(m8[:, NB * h + j:NB * h + j + 1], sl,
                                        mybir.AxisListType.X, maxop)

        if b == 0:
            # tgt half of the projection: PE is free while batch 0's
            # stats resolve; accumulation groups stay open to the end.
            for hh in range(2):
                for k in range(KC // 2):
                    nc.tensor.matmul(po[hh], lhsT=combT[:, k, :],
                                     rhs=wsb[:, k, 512 * hh:512 * (hh + 1)],
                                     start=(k == 0), stop=False,
                                     skip_group_check=True)

        # --- softmax stats on the [1, 4096] score row -----------------
        m1 = stats.tile([1, 1], FP32)
        nc.vector.tensor_reduce(m1, m8, mybir.AxisListType.X, maxop)
        negm = stats.tile([1, 1], FP32)
        nc.vector.tensor_scalar_mul(negm, m1, -1.0)
        zp = stats.tile([1, 1], FP32)
        nc.scalar.activation(zdisc, scores, Exp, bias=negm, accum_out=zp)
        rz = stats.tile([1, 1], FP32)
        nc.vector.reciprocal(rz, zp)
        negmb = stats.tile([128, 1], FP32)
        nc.gpsimd.partition_broadcast(negmb, negm)
        rzb = stats.tile([128, 1], FP32)
        nc.gpsimd.partition_broadcast(rzb, rz)

        # window weights: exp(score + logpw - m) / Z
        wpre = stats.tile([128, 2], FP32)
        nc.vector.tensor_add(wpre, wsc, lpw)
        wexp = stats.tile([128, 2], FP32)
        nc.scalar.activation(wexp, wpre, Exp, bias=negmb)
        wfin = stats.tile([128, 2], FP32)
        nc.vector.tensor_scalar_mul(wfin, wexp, rzb)

        # weighted.T chunks: contract window rows on the PE
        for c in range(8):
            pw = psw.tile([128, 1], FP32)
            nc.tensor.matmul(pw, lhsT=winsb[:, 0, 128 * c:128 * (c + 1)],
                             rhs=wfin[:, 0:1], start=True, stop=False)
            nc.tensor.matmul(pw, lhsT=winsb[:, 1, 128 * c:128 * (c + 1)],
                             rhs=wfin[:, 1:2], start=False, stop=True)
            nc.vector.tensor_copy(combT[:, KC // 2 + c, b:b + 1], pw)

    # weighted half of the projection closes the accumulation groups
    for hh in range(2):
        for k in range(KC // 2, KC):
            nc.tensor.matmul(po[hh], lhsT=combT[:, k, :],
                             rhs=wsb[:, k, 512 * hh:512 * (hh + 1)],
                             start=False, stop=(k == KC - 1),
                             skip_group_check=True)
        ot = outp.tile([BPC, 512], FP32)
        nc.scalar.activation(ot, po[hh], Tanh)
        nc.sync.dma_start(out=out[:, 512 * hh:512 * (hh + 1)], in_=ot)


def build():
    if "nc" in _CACHE:
        return _CACHE["nc"]
    nc = bacc.Bacc("TRN2", target_bir_lowering=False, debug=False,
                   enable_asserts=False, num_devices=N_CORES)
    srcT = nc.dram_tensor("srcT", [BPC, D, S], BF16, kind="ExternalInput").ap()
    tgt16t = nc.dram_tensor("tgt16t", [D, BPC], BF16,
                            kind="ExternalInput").ap()
    tgt = nc.dram_tensor("tgt", [BPC, D], FP32, kind="ExternalInput").ap()
    tgt_t = nc.dram_tensor("tgt_t", [D, BPC], FP32, kind="ExternalInput").ap()
    srcwin = nc.dram_tensor("srcwin", [BPC, WIN, D], FP32,
                            kind="ExternalInput").ap()
    logpw = nc.dram_tensor("logpw", [BPC, 128, 2], FP32,
                           kind="ExternalInput").ap()
    wmat = nc.dram_tensor("wmat", [2 * D, O], FP32, kind="ExternalInput").ap()
    out = nc.dram_tensor("out", [BPC, O], FP32, kind="ExternalOutput").ap()
    with tile.TileContext(nc) as tc:
        _body(tc, out, srcT, tgt16t, tgt, tgt_t, srcwin, logpw, wmat)
    nc.compile()
    _CACHE["nc"] = nc
    return nc


def make_in_maps(src, tgt, pos, wmat):
    """Host-side sharding + bf16 transpose + window/log-posweight precompute."""
    src16 = src.astype(ml_dtypes.bfloat16)
    w0 = np.clip(128 * ((pos.astype(np.int64) - HALF) // 128), 0, S - WIN)
    p_idx = np.arange(128, dtype=np.int64)[:, None]
    t_idx = np.arange(2, dtype=np.int64)[None, :]
    in_maps = []
    for c in range(N_CORES):
        bsl = slice(c * BPC, (c + 1) * BPC)
        srcwin = np.stack([
            src[c * BPC + i, w0[c * BPC + i]:w0[c * BPC + i] + WIN, :]
            for i in range(BPC)
        ])
        logpw = np.stack([
            -((w0[c * BPC + i] + t_idx * 128 + p_idx
               - pos[c * BPC + i]).astype(np.float64) ** 2)
            / (2.0 * STDDEV * STDDEV)
            for i in range(BPC)
        ]).astype(np.float32)
        in_maps.append({
            "srcT": np.ascontiguousarray(src16[bsl].transpose(0, 2, 1)),
            "tgt16t": np.ascontiguousarray(
                tgt[bsl].T.astype(ml_dtypes.bfloat16)),
            "tgt": np.ascontiguousarray(tgt[bsl]),
            "tgt_t": np.ascontiguousarray(tgt[bsl].T),
            "srcwin": np.ascontiguousarray(srcwin),
            "logpw": logpw,
            "wmat": wmat,
        })
    return in_maps


def kernel(source_hidden_sequence, target_hidden, positions,
           attention_weights, trace=False):
    src = np.ascontiguousarray(source_hidden_sequence, dtype=np.float32)
    tgt = np.ascontiguousarray(target_hidden, dtype=np.float32)
    pos = np.asarray(positions)
    wmat = np.ascontiguousarray(attention_weights, dtype=np.float32)
    assert src.shape == (B, S, D) and wmat.shape == (2 * D, O)

    nc = build()
    if trace:
        _install_ntff_shim()
    in_maps = make_in_maps(src, tgt, pos, wmat)
    res = run_bass_kernel_spmd(nc, in_maps, list(range(N_CORES)), trace=trace)
    global LAST_RESULTS
    LAST_RESULTS = res
    out = np.concatenate([res.results[c]["out"] for c in range(N_CORES)],
                         axis=0)
    return out.astype(np.float32)
